# revision 3
# baseline (speedup 1.0000x reference)
"""Trainium2 kernel for nn_BranchingLayer (gnn_message_passing).

Math (with parents_idxs == arange, the spec-guaranteed fill):
    node i = (p, e), p in [0,128), e in [0,8192)
    h      = relu([x_i | g_e] @ W1 + b1)          # [N, 256]
    proj_i = h @ W2 + b2                          # [N, 128]
    children[(p*2+b)*E + e, f] = proj[p*E+e, b*64+f]
    new_x  = concat([x, children], axis=0); event = tile(arange(E), 384)

Device strategy (8 cores, sharded by parents, 16 parents/core):
  - Activations kept feature-major ("transposed space"): SBUF holds
    [128, cols] regions whose partitions are the 128 concat features
    (64 x-features + 64 g-features) and whose columns are events.
    The g half is resident (loaded once); x halves stream in per parent.
  - Two regions: E (x on partitions 0-63, parents 0-7) and O (x on
    partitions 64-127, parents 8-15, with row-flipped W1) so input DMAs
    alternate SBUF port groups (64-partition DMAs only reach half the
    ports).
  - mm1: hT[128,512]x2 = W1half.T @ rhs (fp32r, weights stationary)
  - relu+b1 on ACT (half 0) / DVE (half 1), PSUM -> SBUF, rounded to fp32r
  - mm2: projT[128,512] = W2a.T @ h0 + W2b.T @ h1 (accumulate in PSUM)
  - +b2 copy PSUM -> out staging (alternating ACT/DVE), DMA out via gpsimd
  - Output is projT chunks [128 feats, 4096 events]; host transposes back.

fp32r (TF32-like) matmuls measured at ~2.3e-4 absmax relative error on HW.
"""

import numpy as np

N_CORES = 8
N_EVENTS = 8192
N_PARENTS = 128
NB = 2
NF = 64
HID = 256
PPC = N_PARENTS // N_CORES      # parents per core = 16
HALF = 4096                     # columns per chunk (half a parent's events)
CHUNKS = PPC * 2                # 32 chunks per core
TILE = 512                      # matmul moving free dim
N_NODES = N_PARENTS * N_EVENTS

_module_cache = {}


def _build_module(repeats=1):
    import concourse.tile as tile
    from concourse import bacc, mybir

    F32 = mybir.dt.float32
    F32R = mybir.dt.float32r
    AF = mybir.ActivationFunctionType
    ALU = mybir.AluOpType

    nc = bacc.Bacc("TRN2", target_bir_lowering=False, debug=False,
                   num_devices=N_CORES)

    xt = nc.dram_tensor("xt", [CHUNKS, NF, HALF], F32R, kind="ExternalInput").ap()
    gt = nc.dram_tensor("gt", [NF, N_EVENTS], F32R, kind="ExternalInput").ap()
    w1 = nc.dram_tensor("w1", [128, HID], F32R, kind="ExternalInput").ap()
    w1f = nc.dram_tensor("w1f", [128, HID], F32R, kind="ExternalInput").ap()
    w2a = nc.dram_tensor("w2a", [128, 128], F32R, kind="ExternalInput").ap()
    w2b = nc.dram_tensor("w2b", [128, 128], F32R, kind="ExternalInput").ap()
    b1 = nc.dram_tensor("b1", [128, 2], F32, kind="ExternalInput").ap()
    b2 = nc.dram_tensor("b2", [128, 1], F32, kind="ExternalInput").ap()
    out = nc.dram_tensor("out", [CHUNKS, 128, HALF], F32, kind="ExternalOutput").ap()

    with tile.TileContext(nc) as tc:
        with (
            tc.tile_pool(name="const", bufs=1) as cpool,
            tc.tile_pool(name="regions", bufs=1) as rpool,
            tc.tile_pool(name="hbuf", bufs=4) as hpool,
            tc.tile_pool(name="obuf", bufs=3) as opool,
            tc.tile_pool(name="psh", bufs=4, space="PSUM") as psh,
            tc.tile_pool(name="psp", bufs=2, space="PSUM") as psp,
        ):
            w1_sb = cpool.tile([128, HID], F32R, tag="w1")
            w1f_sb = cpool.tile([128, HID], F32R, tag="w1f")
            w2a_sb = cpool.tile([128, 128], F32R, tag="w2a")
            w2b_sb = cpool.tile([128, 128], F32R, tag="w2b")
            b1_sb = cpool.tile([128, 2], F32, tag="b1")
            b2_sb = cpool.tile([128, 1], F32, tag="b2")
            nc.sync.dma_start(w1_sb[:], w1)
            nc.sync.dma_start(w1f_sb[:], w1f)
            nc.sync.dma_start(w2a_sb[:], w2a)
            nc.sync.dma_start(w2b_sb[:], w2b)
            nc.sync.dma_start(b1_sb[:], b1)
            nc.sync.dma_start(b2_sb[:], b2)

            # Regions: [128, 2*HALF]; column c <-> event c.
            # E: x at partitions 0-63 (parents 0-7), g resident at 64-127.
            # O: x at partitions 64-127 (parents 8-15), g resident at 0-63.
            Ereg = rpool.tile([128, 2 * HALF], F32R, tag="Ereg")
            Oreg = rpool.tile([128, 2 * HALF], F32R, tag="Oreg")
            nc.sync.dma_start(Ereg[64:128, 0:HALF], gt[:, 0:HALF])
            nc.sync.dma_start(Ereg[64:128, HALF:2 * HALF], gt[:, HALF:2 * HALF])
            nc.sync.dma_start(Oreg[0:64, 0:HALF], gt[:, 0:HALF])
            nc.sync.dma_start(Oreg[0:64, HALF:2 * HALF], gt[:, HALF:2 * HALF])

            def body(_iv=None):
                # 32 interleaved steps: E chunk j, O chunk j, E chunk j+1, ...
                for s in range(CHUNKS):
                    j, odd = divmod(s, 2)
                    if not odd:
                        reg, xlo, w1cur, kin = Ereg, 0, w1_sb, j
                    else:
                        reg, xlo, w1cur, kin = Oreg, 64, w1f_sb, PPC + j
                    hc = j % 2
                    c0 = hc * HALF
                    nc.sync.dma_start(reg[xlo:xlo + 64, c0:c0 + HALF], xt[kin])
                    otile = opool.tile([128, HALF], F32, tag="otile")
                    for t in range(HALF // TILE):
                        cs = c0 + t * TILE
                        rhs = reg[:, cs:cs + TILE]
                        ph0 = psh.tile([128, TILE], F32, tag="ph")
                        ph1 = psh.tile([128, TILE], F32, tag="ph")
                        nc.tensor.matmul(ph0[:], w1cur[:, 0:128], rhs,
                                         start=True, stop=True)
                        nc.tensor.matmul(ph1[:], w1cur[:, 128:256], rhs,
                                         start=True, stop=True)
                        h0 = hpool.tile([128, TILE], F32R, tag="h")
                        h1 = hpool.tile([128, TILE], F32R, tag="h")
                        nc.scalar.activation(h0[:], ph0[:], AF.Relu,
                                             bias=b1_sb[:, 0:1])
                        nc.vector.tensor_scalar(h1[:], ph1[:], b1_sb[:, 1:2],
                                                0.0, ALU.add, ALU.max)
                        pp = psp.tile([128, TILE], F32, tag="pp")
                        nc.tensor.matmul(pp[:], w2a_sb[:], h0[:],
                                         start=True, stop=False)
                        nc.tensor.matmul(pp[:], w2b_sb[:], h1[:],
                                         start=False, stop=True)
                        osl = otile[:, t * TILE:(t + 1) * TILE]
                        if t % 2 == 0:
                            nc.scalar.activation(osl, pp[:], AF.Identity,
                                                 bias=b2_sb[:, 0:1])
                        else:
                            nc.vector.tensor_scalar(osl, pp[:], b2_sb[:, 0:1],
                                                    None, ALU.add)
                    nc.gpsimd.dma_start(out[kin], otile[:])

            if repeats == 1:
                body()
            else:
                with tc.For_i(0, repeats, 1) as _i:
                    body(_i)

    nc.compile()
    return nc


def _get_module(repeats=1):
    if repeats not in _module_cache:
        _module_cache[repeats] = _build_module(repeats)
    return _module_cache[repeats]


def _prepare_in_maps(x, g, W1, b1, W2, b2):
    """Host-side shard + layout transform. Returns list of per-core in_maps."""
    # x [N_NODES, 64] -> per-core transposed chunks [32, 64, 4096]
    x4 = x.reshape(N_PARENTS, 2, HALF, NF).transpose(0, 1, 3, 2)  # [p, hc, f, j]
    x4 = x4.reshape(N_CORES, CHUNKS, NF, HALF)
    gt = np.ascontiguousarray(g.T)                                 # [64, 8192]
    w1c = np.ascontiguousarray(W1)                                 # [128, 256]
    w1fc = np.ascontiguousarray(np.concatenate([W1[NF:], W1[:NF]], axis=0))
    w2a = np.ascontiguousarray(W2[:128])
    w2b = np.ascontiguousarray(W2[128:])
    b1p = np.ascontiguousarray(np.stack([b1[:128], b1[128:]], axis=1))  # [128,2]
    b2c = np.ascontiguousarray(b2[:, None])                             # [128,1]
    in_maps = []
    for m in range(N_CORES):
        in_maps.append({
            "xt": np.ascontiguousarray(x4[m]),
            "gt": gt, "w1": w1c, "w1f": w1fc, "w2a": w2a, "w2b": w2b,
            "b1": b1p, "b2": b2c,
        })
    return in_maps


def _assemble(x, core_outs):
    """core_outs[m]: [32, 128, 4096] -> full (new_x, event)."""
    n_new = N_NODES * (1 + NB)
    new_x = np.empty((n_new, NF), dtype=np.float32)
    new_x[:N_NODES] = x
    rows_per_core = PPC * NB * N_EVENTS  # 262144
    for m in range(N_CORES):
        o = core_outs[m].reshape(PPC, 2, NB, NF, HALF)   # [pl, hc, b, f, j]
        o = o.transpose(0, 2, 1, 4, 3)                    # [pl, b, hc, j, f]
        base = N_NODES + m * rows_per_core
        new_x[base:base + rows_per_core] = o.reshape(rows_per_core, NF)
    event = np.tile(np.arange(N_EVENTS, dtype=np.int32), n_new // N_EVENTS)
    return new_x, event


def _reference_numpy(x, g, W1, b1, W2, b2, parents_idxs):
    """Safety-net path for non-arange parents_idxs (never hit by the spec)."""
    E = g.shape[0]
    n_f = x.shape[1]
    n_b = W2.shape[1] // n_f
    n_p = parents_idxs.shape[0] // E
    pg = g[parents_idxs % E]
    pf = x[parents_idxs]
    h = np.concatenate([pf, pg], axis=1) @ W1 + b1
    np.maximum(h, 0.0, out=h)
    proj = h @ W2 + b2
    m = proj.reshape(n_p, E, n_f * n_b).swapaxes(1, 2)
    m = m.reshape(n_p * n_b, n_f, E).swapaxes(1, 2)
    children = m.reshape(n_p * n_b * E, n_f)
    new_x = np.concatenate([x, children], axis=0)
    event = np.tile(np.arange(E, dtype=np.int32), new_x.shape[0] // E)
    return new_x.astype(np.float32), event


def kernel(x, global_features, W1, b1, W2, b2, parents_idxs):
    from concourse import bass_utils

    x = np.asarray(x, dtype=np.float32)
    g = np.asarray(global_features, dtype=np.float32)
    W1 = np.asarray(W1, dtype=np.float32)
    b1 = np.asarray(b1, dtype=np.float32)
    W2 = np.asarray(W2, dtype=np.float32)
    b2 = np.asarray(b2, dtype=np.float32)
    pidx = np.asarray(parents_idxs)

    expected_arange = (
        x.shape == (N_NODES, NF) and g.shape == (N_EVENTS, NF)
        and pidx.shape == (N_NODES,)
        and np.array_equal(pidx, np.arange(N_NODES, dtype=pidx.dtype))
    )
    if not expected_arange:
        return _reference_numpy(x, g, W1, b1, W2, b2, pidx)

    nc = _get_module(1)
    in_maps = _prepare_in_maps(x, g, W1, b1, W2, b2)
    res = bass_utils.run_bass_kernel_spmd(nc, in_maps,
                                          core_ids=list(range(N_CORES)))
    core_outs = [res.results[m]["out"] for m in range(N_CORES)]
    return _assemble(x, core_outs)
